# revision 2
# baseline (speedup 1.0000x reference)
"""MoE expert FFN (CachedKimiExperts) on 8 Trainium2 NeuronCores.

Expert-parallel sharding: core c owns experts [2c, 2c+1].  Routing
(softmax -> top-k -> renormalize) and token gather/scatter run on the
host; each core streams its two experts' weights (pre-transposed,
cast to fp16 on the host) from HBM once and computes

    h   = x_g @ w1[e].T          (gate/up fused, contract over H)
    act = silu(h[:, :I]) * h[:, I:]
    y   = act @ w2[e].T          (contract over I)

for the gathered token block of each expert.  The host applies the
routing weights and scatter-adds the per-expert outputs.

Shapes (hardcoded): T=256 tokens arbitrary, E=16 experts, H=2048,
I=1024, top_k arbitrary.  Device kernel is built for a token capacity
C (multiple of 128) covering the largest expert batch.
"""

import os
import sys

import numpy as np

for _p in ("/opt/trn_rl_repo", "/root/.axon_site/_ro/trn_rl_repo"):
    if os.path.isdir(_p) and _p not in sys.path:
        sys.path.append(_p)

import concourse.bass as bass  # noqa: F401  (bass must import before tile)
import concourse.mybir as mybir
import concourse.tile as tile
from concourse import bacc, bass_utils
from concourse.masks import make_identity

N_CORES = 8
E = 16
E_LOC = E // N_CORES  # experts per core
H = 2048  # hidden dim
I = 1024  # expert intermediate dim
I2 = 2 * I  # fused gate+up width
P = 128  # partitions
FD = 512  # matmul moving free dim (one fp32 PSUM bank)

F16 = mybir.dt.float16
F32 = mybir.dt.float32

TRACE = False
TRACE_CORES = None
LAST_RESULTS = None

_programs = {}


def _build_program(C):
    """Bass/Tile program for one core: E_LOC experts x (C tokens each)."""
    KC1 = H // P  # k-chunks for the gate/up matmul (contract over H)
    KC2 = I // P  # k-chunks for the down matmul (contract over I)
    NH = H // 2  # down-proj output processed in two halves (PSUM budget)
    CB = C // P  # token blocks per expert

    nc = bacc.Bacc(
        "TRN2", target_bir_lowering=False, debug=False, num_devices=N_CORES
    )
    w1t = nc.dram_tensor("w1t", [E_LOC, H, I2], F16, kind="ExternalInput")
    w2t = nc.dram_tensor("w2t", [E_LOC, I, H], F16, kind="ExternalInput")
    xg = nc.dram_tensor("xg", [E_LOC, KC1, P, C], F16, kind="ExternalInput")
    y = nc.dram_tensor("y", [E_LOC, C, H], F32, kind="ExternalOutput")

    with tile.TileContext(nc) as tc:
        with (
            tc.tile_pool(name="w1p", bufs=6) as w1p,
            tc.tile_pool(name="w2p", bufs=6) as w2p,
            tc.tile_pool(name="xp", bufs=2) as xp,
            tc.tile_pool(name="actp", bufs=2) as actp,
            tc.tile_pool(name="yp", bufs=3) as yp,
            tc.tile_pool(name="constp", bufs=1) as constp,
            tc.tile_pool(name="pgu", bufs=2, space="PSUM") as pgu,
            tc.tile_pool(name="ptp", bufs=2, space="PSUM") as ptp,
            tc.tile_pool(name="py", bufs=1, space="PSUM") as py,
        ):
            ident = constp.tile([P, P], F16, name="ident")
            make_identity(nc, ident)

            for e in range(E_LOC):
                for cb in range(CB):
                    # gathered tokens for this block: [H(part-chunked), P]
                    xg_t = xp.tile([P, KC1, P], F16, tag="xg", name="xg_t")
                    nc.sync.dma_start(
                        xg_t,
                        xg[e, :, :, cb * P : (cb + 1) * P].rearrange(
                            "k p c -> p k c"
                        ),
                    )

                    # ---- gate/up projection: h[c, i2] = x @ w1[e].T ----
                    gate_ps = pgu.tile([P, I], F32, tag="gu", name="gate_ps")
                    up_ps = pgu.tile([P, I], F32, tag="gu", name="up_ps")
                    for kc in range(KC1):
                        w1_t = w1p.tile([P, I2], F16, tag="w1", name="w1_t")
                        nc.sync.dma_start(w1_t, w1t[e, kc * P : (kc + 1) * P, :])
                        lhsT = xg_t[:, kc, :]
                        st = kc == 0
                        sp = kc == KC1 - 1
                        for nb in range(I // FD):
                            nc.tensor.matmul(
                                gate_ps[:, nb * FD : (nb + 1) * FD],
                                lhsT,
                                w1_t[:, nb * FD : (nb + 1) * FD],
                                start=st,
                                stop=sp,
                            )
                        for nb in range(I // FD):
                            nc.tensor.matmul(
                                up_ps[:, nb * FD : (nb + 1) * FD],
                                lhsT,
                                w1_t[:, I + nb * FD : I + (nb + 1) * FD],
                                start=st,
                                stop=sp,
                            )

                    # ---- act = silu(gate) * up, cast to fp16 ----
                    sg = actp.tile([P, I], F32, tag="sg", name="sg")
                    nc.scalar.activation(
                        sg, gate_ps, mybir.ActivationFunctionType.Silu
                    )
                    act = actp.tile([P, I], F16, tag="act", name="act")
                    nc.vector.tensor_mul(act, sg, up_ps)

                    # ---- transpose act -> [I(part-chunked), tokens] ----
                    tp_ps = ptp.tile([P, KC2, P], F16, tag="tp", name="tp_ps")
                    for j in range(KC2):
                        nc.tensor.transpose(
                            tp_ps[:, j, :], act[:, j * P : (j + 1) * P], ident
                        )
                    actT = actp.tile([P, KC2, P], F16, tag="actT", name="actT")
                    nc.vector.tensor_copy(actT, tp_ps)

                    # ---- down projection: y[c, h] = act @ w2[e].T ----
                    for hh in range(2):
                        y_ps = py.tile([P, NH], F32, tag="y", name="y_ps")
                        for j in range(KC2):
                            w2_t = w2p.tile([P, NH], F16, tag="w2", name="w2_t")
                            nc.sync.dma_start(
                                w2_t,
                                w2t[
                                    e,
                                    j * P : (j + 1) * P,
                                    hh * NH : (hh + 1) * NH,
                                ],
                            )
                            for nb in range(NH // FD):
                                nc.tensor.matmul(
                                    y_ps[:, nb * FD : (nb + 1) * FD],
                                    actT[:, j, :],
                                    w2_t[:, nb * FD : (nb + 1) * FD],
                                    start=(j == 0),
                                    stop=(j == KC2 - 1),
                                )
                        y_sb = yp.tile([P, NH], F32, tag="ysb", name="y_sb")
                        nc.vector.tensor_copy(y_sb, y_ps)
                        nc.sync.dma_start(
                            y[
                                e,
                                cb * P : (cb + 1) * P,
                                hh * NH : (hh + 1) * NH,
                            ],
                            y_sb,
                        )
    nc.finalize()
    return nc


def _route(router_logits, top_k):
    """softmax -> top-k -> renormalize; per-expert token lists + weights."""
    lg = np.asarray(router_logits, dtype=np.float64)
    T, num_e = lg.shape
    k = int(np.asarray(top_k))
    p = np.exp(lg - lg.max(axis=-1, keepdims=True))
    p /= p.sum(axis=-1, keepdims=True)
    idx = np.argpartition(-p, k - 1, axis=1)[:, :k]  # [T, k] top-k set
    vals = np.take_along_axis(p, idx, axis=1)
    wts = vals / vals.sum(axis=-1, keepdims=True)
    tok_idx = [[] for _ in range(num_e)]
    tok_w = [[] for _ in range(num_e)]
    for t in range(T):
        for j in range(k):
            tok_idx[idx[t, j]].append(t)
            tok_w[idx[t, j]].append(wts[t, j])
    return tok_idx, tok_w


def kernel(x, router_logits, w1, w2, top_k):
    global LAST_RESULTS
    x = np.asarray(x)
    w1 = np.asarray(w1)
    w2 = np.asarray(w2)
    T = x.shape[0]

    tok_idx, tok_w = _route(router_logits, top_k)
    max_count = max(max(len(ti) for ti in tok_idx), 1)
    C = ((max_count + P - 1) // P) * P

    prog = _programs.get(C)
    if prog is None:
        prog = _programs[C] = _build_program(C)

    xT16 = np.ascontiguousarray(x.T.astype(np.float16))  # [H, T]
    in_maps = []
    for c in range(N_CORES):
        sl = slice(c * E_LOC, (c + 1) * E_LOC)
        w1tc = np.ascontiguousarray(
            w1[sl].transpose(0, 2, 1).astype(np.float16)
        )  # [E_LOC, H, 2I]
        w2tc = np.ascontiguousarray(
            w2[sl].transpose(0, 2, 1).astype(np.float16)
        )  # [E_LOC, I, H]
        xgc = np.zeros((E_LOC, H // P, P, C), np.float16)
        for el in range(E_LOC):
            ti = tok_idx[c * E_LOC + el]
            if ti:
                xgc[el, :, :, : len(ti)] = xT16[:, ti].reshape(
                    H // P, P, len(ti)
                )
        in_maps.append({"w1t": w1tc, "w2t": w2tc, "xg": xgc})

    LAST_RESULTS = bass_utils.run_bass_kernel_spmd(
        prog,
        in_maps,
        core_ids=list(range(N_CORES)),
        trace=TRACE,
        trace_cores=TRACE_CORES,
    )

    out = np.zeros((T, H), dtype=np.float64)
    for c in range(N_CORES):
        yv = LAST_RESULTS.results[c]["y"]  # [E_LOC, C, H] fp32
        for el in range(E_LOC):
            ge = c * E_LOC + el
            ti = tok_idx[ge]
            if ti:
                wv = np.asarray(tok_w[ge], dtype=np.float64)[:, None]
                out[ti] += wv * yv[el][: len(ti)].astype(np.float64)
    return out.astype(x.dtype)


# revision 3
# speedup vs baseline: 1.1486x; 1.1486x over previous
"""MoE expert FFN (CachedKimiExperts) on 8 Trainium2 NeuronCores.

Expert-parallel sharding: core c owns experts [2c, 2c+1].  Routing
(softmax -> top-k -> renormalize) and token gather/scatter run on the
host; each core streams its two experts' weights (pre-transposed,
cast to fp16 on the host) from HBM once and computes

    h   = x_g @ w1[e].T          (gate/up fused, contract over H)
    act = silu(h[:, :I]) * h[:, I:]
    y   = act @ w2[e].T          (contract over I)

for the gathered token block of each expert.  The host applies the
routing weights and scatter-adds the per-expert outputs.

Matmul layout: gathered tokens (transposed) are the stationary operand,
weights stream through the PE at N=512; weight tiles are 1 MiB (two
128-row k-chunks paired in the free dim, prepared host-side) so DMA
descriptor-issue on SyncE stays well ahead of the 16 DMA engines.
"""

import os
import sys

import numpy as np

for _p in ("/opt/trn_rl_repo", "/root/.axon_site/_ro/trn_rl_repo"):
    if os.path.isdir(_p) and _p not in sys.path:
        sys.path.append(_p)

import concourse.bass as bass  # noqa: F401  (bass must import before tile)
import concourse.mybir as mybir
import concourse.tile as tile
from concourse import bacc, bass_utils
from concourse.masks import make_identity

N_CORES = 8
E = 16
E_LOC = E // N_CORES  # experts per core
H = 2048  # hidden dim
I = 1024  # expert intermediate dim
I2 = 2 * I  # fused gate+up width
P = 128  # partitions
FD = 512  # matmul moving free dim (one fp32 PSUM bank)

F16 = mybir.dt.float16
F32 = mybir.dt.float32

TRACE = False
TRACE_CORES = None
LAST_RESULTS = None

_programs = {}


def _build_program(C):
    """Bass/Tile program for one core: E_LOC experts x (C tokens each)."""
    KC1 = H // P  # k-chunks for the gate/up matmul (contract over H)
    KC2 = I // P  # k-chunks for the down matmul (contract over I)
    G1 = KC1 // 2  # w1 tile groups (2 k-chunks per 1MiB tile)
    G2 = KC2 // 2  # w2 tile groups
    CB = C // P  # token blocks per expert
    NH = H // 2

    nc = bacc.Bacc(
        "TRN2", target_bir_lowering=False, debug=False, num_devices=N_CORES
    )
    # w1p[e, g, p, j, :] = w1[2c+e].T[(2g+j)*128 + p, :]   (gate/up fused)
    w1p = nc.dram_tensor("w1p", [E_LOC, G1, P, 2, I2], F16, kind="ExternalInput")
    # w2p[e, g, p, j, :] = w2[2c+e].T[(2g+j)*128 + p, :]
    w2p = nc.dram_tensor("w2p", [E_LOC, G2, P, 2, H], F16, kind="ExternalInput")
    # xg[e, p, kc, c] = x.T[kc*128 + p, tok_c(e)]  (gathered, padded)
    xg = nc.dram_tensor("xg", [E_LOC, P, KC1, C], F16, kind="ExternalInput")
    y = nc.dram_tensor("y", [E_LOC, C, H], F16, kind="ExternalOutput")

    with tile.TileContext(nc) as tc:
        with (
            tc.tile_pool(name="w1pool", bufs=4) as w1pool,
            tc.tile_pool(name="w2pool", bufs=3) as w2pool,
            tc.tile_pool(name="xp", bufs=2) as xp,
            tc.tile_pool(name="actp", bufs=2) as actp,
            tc.tile_pool(name="yp", bufs=4) as yp,
            tc.tile_pool(name="constp", bufs=1) as constp,
            tc.tile_pool(name="pgu", bufs=2, space="PSUM") as pgu,
            tc.tile_pool(name="py", bufs=1, space="PSUM") as py,
        ):
            ident = constp.tile([P, P], F16, name="ident")
            make_identity(nc, ident)

            for e in range(E_LOC):
                for cb in range(CB):
                    xg_t = xp.tile([P, KC1, P], F16, tag="xg", name="xg_t")
                    nc.sync.dma_start(
                        xg_t, xg[e, :, :, cb * P : (cb + 1) * P]
                    )

                    # ---- gate/up projection: h[c, i2] = x @ w1[e].T ----
                    gate_ps = pgu.tile([P, I], F32, tag="gu", name="gate_ps")
                    up_ps = pgu.tile([P, I], F32, tag="gu", name="up_ps")
                    for g in range(G1):
                        w1_t = w1pool.tile(
                            [P, 2, I2], F16, tag="w1", name="w1_t"
                        )
                        nc.sync.dma_start(w1_t, w1p[e, g])
                        for j in range(2):
                            kc = 2 * g + j
                            lhsT = xg_t[:, kc, :]
                            st = kc == 0
                            sp = kc == KC1 - 1
                            for nb in range(I // FD):
                                nc.tensor.matmul(
                                    gate_ps[:, nb * FD : (nb + 1) * FD],
                                    lhsT,
                                    w1_t[:, j, nb * FD : (nb + 1) * FD],
                                    start=st,
                                    stop=sp,
                                )
                            for nb in range(I // FD):
                                nc.tensor.matmul(
                                    up_ps[:, nb * FD : (nb + 1) * FD],
                                    lhsT,
                                    w1_t[:, j, I + nb * FD : I + (nb + 1) * FD],
                                    start=st,
                                    stop=sp,
                                )

                    # ---- act = silu(gate) * up, cast to fp16 ----
                    sg = actp.tile([P, I], F32, tag="sg", name="sg")
                    nc.scalar.activation(
                        sg, gate_ps, mybir.ActivationFunctionType.Silu
                    )
                    act = actp.tile([P, I], F16, tag="act", name="act")
                    nc.vector.tensor_mul(act, sg, up_ps)

                    # ---- transpose act -> [I(part-chunked), tokens] ----
                    # (shares the gu pool's slots; fp16 -> one PSUM bank)
                    tp_ps = pgu.tile([P, KC2, P], F16, tag="gu", name="tp_ps")
                    for j in range(KC2):
                        nc.tensor.transpose(
                            tp_ps[:, j, :], act[:, j * P : (j + 1) * P], ident
                        )
                    actT = actp.tile([P, KC2, P], F16, tag="actT", name="actT")
                    nc.vector.tensor_copy(actT, tp_ps)

                    # ---- down projection: y[c, h] = act @ w2[e].T ----
                    y_ps = py.tile([P, H], F32, tag="y", name="y_ps")
                    for g in range(G2):
                        w2_t = w2pool.tile([P, 2, H], F16, tag="w2", name="w2_t")
                        nc.sync.dma_start(w2_t, w2p[e, g])
                        for j in range(2):
                            ic = 2 * g + j
                            for nb in range(H // FD):
                                nc.tensor.matmul(
                                    y_ps[:, nb * FD : (nb + 1) * FD],
                                    actT[:, ic, :],
                                    w2_t[:, j, nb * FD : (nb + 1) * FD],
                                    start=(ic == 0),
                                    stop=(ic == KC2 - 1),
                                )
                    for hh in range(2):
                        y_sb = yp.tile([P, NH], F16, tag="ysb", name="y_sb")
                        nc.vector.tensor_copy(
                            y_sb, y_ps[:, hh * NH : (hh + 1) * NH]
                        )
                        nc.scalar.dma_start(
                            y[
                                e,
                                cb * P : (cb + 1) * P,
                                hh * NH : (hh + 1) * NH,
                            ],
                            y_sb,
                        )
    nc.finalize()
    return nc


def _route(router_logits, top_k):
    """softmax -> top-k -> renormalize; per-expert token lists + weights."""
    lg = np.asarray(router_logits, dtype=np.float64)
    T, num_e = lg.shape
    k = int(np.asarray(top_k))
    p = np.exp(lg - lg.max(axis=-1, keepdims=True))
    p /= p.sum(axis=-1, keepdims=True)
    idx = np.argpartition(-p, k - 1, axis=1)[:, :k]  # [T, k] top-k set
    vals = np.take_along_axis(p, idx, axis=1)
    wts = vals / vals.sum(axis=-1, keepdims=True)
    tok_idx = [[] for _ in range(num_e)]
    tok_w = [[] for _ in range(num_e)]
    for t in range(T):
        for j in range(k):
            tok_idx[idx[t, j]].append(t)
            tok_w[idx[t, j]].append(wts[t, j])
    return tok_idx, tok_w


def kernel(x, router_logits, w1, w2, top_k):
    global LAST_RESULTS
    x = np.asarray(x)
    w1 = np.asarray(w1)
    w2 = np.asarray(w2)
    T = x.shape[0]

    tok_idx, tok_w = _route(router_logits, top_k)
    max_count = max(max(len(ti) for ti in tok_idx), 1)
    C = ((max_count + P - 1) // P) * P

    prog = _programs.get(C)
    if prog is None:
        prog = _programs[C] = _build_program(C)

    KC1 = H // P
    xT16 = np.ascontiguousarray(x.T.astype(np.float16))  # [H, T]
    in_maps = []
    for c in range(N_CORES):
        sl = slice(c * E_LOC, (c + 1) * E_LOC)
        # [E_LOC, H, 2I] -> [E_LOC, G1, P, 2, I2] (pair k-chunks in free dim)
        w1tc = w1[sl].transpose(0, 2, 1).astype(np.float16)
        w1pc = np.ascontiguousarray(
            w1tc.reshape(E_LOC, KC1 // 2, 2, P, I2).transpose(0, 1, 3, 2, 4)
        )
        w2tc = w2[sl].transpose(0, 2, 1).astype(np.float16)  # [E_LOC, I, H]
        w2pc = np.ascontiguousarray(
            w2tc.reshape(E_LOC, I // P // 2, 2, P, H).transpose(0, 1, 3, 2, 4)
        )
        xgc = np.zeros((E_LOC, P, KC1, C), np.float16)
        for el in range(E_LOC):
            ti = tok_idx[c * E_LOC + el]
            if ti:
                # [H, n] -> [KC1, P, n] -> [P, KC1, n]
                xgc[el, :, :, : len(ti)] = (
                    xT16[:, ti].reshape(KC1, P, len(ti)).transpose(1, 0, 2)
                )
        in_maps.append({"w1p": w1pc, "w2p": w2pc, "xg": xgc})

    LAST_RESULTS = bass_utils.run_bass_kernel_spmd(
        prog,
        in_maps,
        core_ids=list(range(N_CORES)),
        trace=TRACE,
        trace_cores=TRACE_CORES,
    )

    out = np.zeros((T, H), dtype=np.float64)
    for c in range(N_CORES):
        yv = LAST_RESULTS.results[c]["y"]  # [E_LOC, C, H] fp16
        for el in range(E_LOC):
            ge = c * E_LOC + el
            ti = tok_idx[ge]
            if ti:
                wv = np.asarray(tok_w[ge], dtype=np.float64)[:, None]
                out[ti] += wv * yv[el][: len(ti)].astype(np.float64)
    return out.astype(x.dtype)
